# revision 1
# baseline (speedup 1.0000x reference)
"""Trainium2 Bass kernel for the soft-decision-tree ensemble problem.

Math (per reference):
  I = onehot(argmax_d T[e,n,:]) ; t = max_d T[e,n,:]
  s[b,en] = floor(t[en] - x[b, argmax_d])
  p[b,e,l] = prod_j (bit ? 1-s : s) over the leaf's 6 ancestors
  out = softmax(p @ L, axis=classes)

Strategy: data-parallel over the batch across 8 cores (1024 rows each).
T/L replicated. Everything on device; selection done as a one-hot matmul
with an augmented K row carrying +t so PSUM holds u = t - x_dot directly.
floor via u - python_mod(u, 1). Tree products as fused scalar_tensor_tensor
ops. Final matmul in fp32, stable softmax on ACT/DVE.
"""
import os
import sys

for p in ("/opt/trn_rl_repo",):
    if p not in sys.path and os.path.isdir(p):
        sys.path.insert(0, p)

import numpy as np
import ml_dtypes

import concourse.bass as bass
import concourse.tile as tile
from concourse import bacc, mybir
from concourse.bass_utils import run_bass_kernel_spmd

# problem constants (hardcoded per contract)
B, D = 8192, 512
E, NN, NL, C = 16, 63, 64, 100
DEPTH = 6
NCORES = 8
BC = B // NCORES          # rows per core = 1024
CH = BC // 128            # 128-row chunks per core = 8
EN = E * NN               # 1008
HALF = EN // 2            # 504 (= 8 estimators * 63 nodes)

F32 = mybir.dt.float32
BF16 = mybir.dt.bfloat16

# selection matmul dtype: "fp32" (exact, 4 cyc/row) or "fp32r" (fast, exactness TBD)
SEL_DTYPE = os.environ.get("KERNEL_SEL_DTYPE", "fp32")


def build_program():
    nc = bacc.Bacc(
        "TRN2",
        target_bir_lowering=False,
        debug=False,
        enable_asserts=False,
        num_devices=NCORES,
    )

    x_in = nc.dram_tensor("x", [BC, D], F32, kind="ExternalInput").ap()
    T_in = nc.dram_tensor("T", [E, NN, D], F32, kind="ExternalInput").ap()
    L_in = nc.dram_tensor("L", [E, NL, C], F32, kind="ExternalInput").ap()
    idf_in = nc.dram_tensor("idf", [128, 128], F32, kind="ExternalInput").ap()
    idb_in = nc.dram_tensor("idb", [128, 128], BF16, kind="ExternalInput").ap()
    out_d = nc.dram_tensor("out", [BC, C], F32, kind="ExternalOutput").ap()
    t_scratch = nc.dram_tensor("t_scratch", [EN], F32).ap()

    sel_dt = F32 if SEL_DTYPE == "fp32" else mybir.dt.float32r

    with tile.TileContext(nc) as tc:
        with (
            tc.tile_pool(name="const", bufs=1) as constp,
            tc.tile_pool(name="tproc", bufs=1) as tprocp,
            tc.tile_pool(name="big", bufs=1) as bigp,
            tc.tile_pool(name="work", bufs=2) as workp,
            tc.tile_pool(name="psum", bufs=3, space="PSUM") as psump,
            tc.tile_pool(name="psum_mm", bufs=2, space="PSUM") as psummp,
        ):
            # ---- constants ----
            idf = constp.tile([128, 128], F32)
            nc.sync.dma_start(idf[:], idf_in[:])
            idb = constp.tile([128, 128], BF16)
            nc.sync.dma_start(idb[:], idb_in[:])
            ones = constp.tile([1, 128], F32)
            nc.vector.memset(ones[:], 1.0)

            # ---- T processing ----
            # T_sb[p, t, d] = T[en = t*126 + p, d]; tile t covers estimators 2t, 2t+1
            T_sb = tprocp.tile([126, 8, D], F32)
            nc.sync.dma_start(
                T_sb[:], T_in.rearrange("e n d -> (e n) d").rearrange("(t p) d -> p t d", p=126)
            )
            tmax = tprocp.tile([126, 8], F32)
            nc.vector.tensor_reduce(
                tmax[:], T_sb[:], axis=mybir.AxisListType.X, op=mybir.AluOpType.max
            )
            I_sb = tprocp.tile([126, 8, D], BF16)
            for t in range(8):
                nc.vector.tensor_scalar(
                    I_sb[:, t, :], T_sb[:, t, :], tmax[:, t : t + 1], None,
                    op0=mybir.AluOpType.is_equal,
                )
            # t_row: [126,8] -> dram [1008] -> sbuf [1,1008]
            nc.sync.dma_start(t_scratch.rearrange("(t p) -> p t", p=126), tmax[:])
            t_row = constp.tile([1, EN], F32)
            nc.sync.dma_start(t_row[:1, :], t_scratch.rearrange("(o x) -> o x", o=1))

            # I^T: [128 d-part, 4 d-chunk, 1008 en] fp32 (0/1 values)
            I_dT = bigp.tile([128, 4, EN], sel_dt)
            for t in range(8):
                for c in range(4):
                    tp = psump.tile([128, 126], BF16, tag="tp")
                    nc.tensor.transpose(
                        tp[:], I_sb[:, t, c * 128 : (c + 1) * 128], idb[:126, :126]
                    )
                    nc.scalar.activation(
                        I_dT[:, c, t * 126 : (t + 1) * 126], tp[:],
                        mybir.ActivationFunctionType.Copy,
                    )

            # ---- x load + transpose (negated) ----
            x_sb = bigp.tile([128, CH, D], F32)
            nc.sync.dma_start(x_sb[:], x_in.rearrange("(k p) d -> p k d", p=128))
            xTn = bigp.tile([128, 4, BC], sel_dt)
            for k in range(CH):
                for c in range(4):
                    tp = psump.tile([128, 128], F32, tag="tp")
                    nc.tensor.transpose(
                        tp[:], x_sb[:, k, c * 128 : (c + 1) * 128], idf[:]
                    )
                    nc.scalar.activation(
                        xTn[:, c, k * 128 : (k + 1) * 128], tp[:],
                        mybir.ActivationFunctionType.Copy, scale=-1.0,
                    )

            # ---- selection matmul + floor ----
            # u = t - x_dot accumulated in PSUM; w = pymod(u,1) - u = -floor(u) = -s
            ones_sel = ones
            t_row_sel = t_row
            if SEL_DTYPE != "fp32":
                ones_sel = constp.tile([1, 128], sel_dt)
                nc.vector.memset(ones_sel[:], 1.0)
                t_row_sel = constp.tile([1, EN], sel_dt)
                nc.vector.tensor_copy(t_row_sel[:], t_row[:])
            w_sb = bigp.tile([128, CH, EN], BF16)  # -s: small ints, exact in bf16
            for k in range(CH):
                for h in range(2):
                    u_ps = psummp.tile([128, HALF], F32, tag="mm")
                    for c in range(4):
                        nc.tensor.matmul(
                            u_ps[:],
                            lhsT=xTn[:, c, k * 128 : (k + 1) * 128],
                            rhs=I_dT[:, c, h * HALF : (h + 1) * HALF],
                            start=(c == 0), stop=False,
                        )
                    nc.tensor.matmul(
                        u_ps[:],
                        lhsT=ones_sel[:1, :],
                        rhs=t_row_sel[:1, h * HALF : (h + 1) * HALF],
                        start=False, stop=True,
                    )
                    # floor(u) = round(u) - [round(u) > u]  (cast rounds to nearest)
                    # w = -floor(u) = flag - round(u)
                    ri = workp.tile([128, HALF], mybir.dt.int32, tag="ri")
                    nc.vector.tensor_copy(ri[:], u_ps[:])
                    flag = workp.tile([128, HALF], F32, tag="flag")
                    nc.vector.scalar_tensor_tensor(
                        flag[:], ri[:], 0.0, u_ps[:],
                        op0=mybir.AluOpType.add, op1=mybir.AluOpType.is_gt,
                    )
                    nc.vector.tensor_tensor(
                        w_sb[:, k, h * HALF : (h + 1) * HALF], flag[:], ri[:],
                        op=mybir.AluOpType.subtract,
                    )

            # ---- tree products ----
            # w4[p, k, e, n]; node n of level j: n in [2^(j-1)-1, 2^j-2]
            w4 = w_sb[:].rearrange("p k (e n) -> p k e n", n=NN)
            lvl = bigp.tile([128, CH, E, 2], F32, tag="lvlA")
            nc.vector.tensor_scalar(
                lvl[:, :, :, 0:1], w4[:, :, :, 0:1], -1.0, None, op0=mybir.AluOpType.mult
            )
            nc.vector.tensor_scalar(
                lvl[:, :, :, 1:2], w4[:, :, :, 0:1], 1.0, None, op0=mybir.AluOpType.add
            )
            for j in range(2, DEPTH + 1):
                half = 2 ** (j - 1)
                base = half - 1
                nxt = bigp.tile([128, CH, E, 2 * half], F32, tag=("lvlA" if j % 2 else "lvlB"))
                nxt5 = nxt[:].rearrange("p k e (k2 c) -> p k e k2 c", c=2)
                wj = w4[:, :, :, base : base + half]
                par = lvl[:]
                # c=0: s * parent = (-w) * parent ; c=1: (1-s)*parent = (1+w)*parent
                nc.vector.scalar_tensor_tensor(
                    nxt5[:, :, :, :, 0], wj, -1.0, par,
                    op0=mybir.AluOpType.mult, op1=mybir.AluOpType.mult,
                )
                nc.vector.scalar_tensor_tensor(
                    nxt5[:, :, :, :, 1], wj, 1.0, par,
                    op0=mybir.AluOpType.add, op1=mybir.AluOpType.mult,
                )
                lvl = nxt
            p_sb = lvl  # [128, CH, E, NL]
            p_flat = p_sb[:].rearrange("p k e l -> p k (e l)")

            # ---- transpose p ----
            pT = bigp.tile([128, CH, BC], F32)  # [el%128, el-chunk j, b]
            for k in range(CH):
                for j in range(CH):
                    tp = psump.tile([128, 128], F32, tag="tp")
                    nc.tensor.transpose(
                        tp[:], p_flat[:, k, j * 128 : (j + 1) * 128], idf[:]
                    )
                    nc.scalar.activation(
                        pT[:, j, k * 128 : (k + 1) * 128], tp[:],
                        mybir.ActivationFunctionType.Copy,
                    )

            # ---- final matmul + softmax ----
            L_sb = constp.tile([128, CH, C], F32)
            nc.sync.dma_start(
                L_sb[:], L_in.rearrange("e l c -> (e l) c").rearrange("(j p) c -> p j c", p=128)
            )
            out_v = out_d.rearrange("(k p) c -> p k c", p=128)
            for k in range(CH):
                y_ps = psummp.tile([128, C], F32, tag="mm")
                for j in range(CH):
                    nc.tensor.matmul(
                        y_ps[:],
                        lhsT=pT[:, j, k * 128 : (k + 1) * 128],
                        rhs=L_sb[:, j, :],
                        start=(j == 0), stop=(j == CH - 1),
                    )
                nm = workp.tile([128, 1], F32, tag="nm")
                nc.vector.tensor_reduce(
                    nm[:], y_ps[:], axis=mybir.AxisListType.X,
                    op=mybir.AluOpType.max, negate=True,
                )
                yexp = workp.tile([128, C], F32, tag="yexp")
                ssum = workp.tile([128, 1], F32, tag="ssum")
                nc.scalar.activation(
                    yexp[:], y_ps[:], mybir.ActivationFunctionType.Exp,
                    bias=nm[:, 0:1], scale=1.0, accum_out=ssum[:, 0:1],
                )
                rec = workp.tile([128, 1], F32, tag="rec")
                nc.vector.reciprocal(rec[:], ssum[:])
                yout = workp.tile([128, C], F32, tag="yout")
                nc.vector.tensor_scalar(
                    yout[:], yexp[:], rec[:, 0:1], None, op0=mybir.AluOpType.mult
                )
                nc.sync.dma_start(out_v[:, k, :], yout[:])

    nc.compile()
    return nc


_id_f32 = np.eye(128, dtype=np.float32)
_id_bf16 = np.eye(128, dtype=ml_dtypes.bfloat16)


def make_in_maps(x, T, L):
    x = np.ascontiguousarray(x, dtype=np.float32)
    T = np.ascontiguousarray(T, dtype=np.float32)
    L = np.ascontiguousarray(L, dtype=np.float32)
    maps = []
    for i in range(NCORES):
        maps.append({
            "x": x[i * BC : (i + 1) * BC],
            "T": T,
            "L": L,
            "idf": _id_f32,
            "idb": _id_bf16,
        })
    return maps


def run(x, T, L, trace=False, **kw):
    nc = build_program()
    res = run_bass_kernel_spmd(
        nc, make_in_maps(x, T, L), core_ids=list(range(NCORES)), trace=trace, **kw
    )
    out = np.concatenate([res.results[i]["out"] for i in range(NCORES)], axis=0)
    return out, res


def kernel(x, T, L):
    out, _ = run(x, T, L, trace=False)
    return out



# revision 22
# speedup vs baseline: 1.6146x; 1.6146x over previous
"""Trainium2 Bass kernel for the soft-decision-tree ensemble problem.

Math (per reference):
  I = onehot(argmax_d entmax15(T)[e,n,:]) ; t[en] = T[e,n,argmax] (= max)
  u[b,en] = t[en] - x[b, argmax] ; s = floor(u)
  p[b,e,l] = prod_j (bit ? 1-s : s) over the leaf's 6 ancestors
  out = softmax(p @ L, axis=classes)

Strategy: data-parallel over the batch across 8 cores (1024 rows each),
T/L replicated. Each x shard is laid out transposed in DRAM ([D, BC]) so
the selection lhsT needs no on-device transpose. Selection runs as TWO
bf16 matmul passes with x split into an exact bf16 hi/lo pair (validated
flip-free on this data), accumulating into PSUM preloaded with -t by the
scalar engine. floor via one DVE op: w = (u mod 1) - u = -floor(u),
emitted as int16. The tree products run in int16 on DVE (max |level|
product is 8000 < 32767), last level emits fp32 p. p is transposed on
the PE and the final matmul + softmax run in fp32.
"""
import os
import sys

for p in ("/opt/trn_rl_repo",):
    if p not in sys.path and os.path.isdir(p):
        sys.path.insert(0, p)

import numpy as np
import ml_dtypes

import concourse.bass as bass
import concourse.tile as tile
from concourse import bacc, mybir
from concourse.bass_utils import run_bass_kernel_spmd

# problem constants (hardcoded per contract)
B, D = 8192, 512
E, NN, NL, C = 16, 63, 64, 100
DEPTH = 6
NCORES = 8
BC = B // NCORES          # rows per core = 1024
CH = BC // 128            # 128-row chunks per core = 8
EN = E * NN               # 1008
HALF = EN // 2            # 504
TT = 8                    # t-tiles of 126 rows (2 estimators each)
EL = E * NL               # 1024

F32 = mybir.dt.float32
F32R = mybir.dt.float32r
BF16 = mybir.dt.bfloat16
I16 = mybir.dt.int16

# p-transpose dtype: "f32" (2 cyc/row, exact) or "f32r" (1.5 cyc/row, if HW-exact)
PT_MODE = os.environ.get("KERNEL_PT", "f32")
# selection: "pair" (2x bf16, exact-validated) or "f32r" (1 pass, if HW-exact)
SEL_MODE = os.environ.get("KERNEL_SEL", "pair")
DEBUG_DUMP = os.environ.get("KERNEL_DEBUG", "") == "1"


def build_program():
    nc = bacc.Bacc(
        "TRN2",
        target_bir_lowering=False,
        debug=False,
        enable_asserts=False,
        num_devices=NCORES,
    )

    xT_in = nc.dram_tensor("xT", [D, BC], F32, kind="ExternalInput").ap()
    T_in = nc.dram_tensor("T", [E, NN, D], F32, kind="ExternalInput").ap()
    L_in = nc.dram_tensor("L", [E, NL, C], F32, kind="ExternalInput").ap()
    idf_in = nc.dram_tensor("idf", [128, 128], F32, kind="ExternalInput").ap()
    idb_in = nc.dram_tensor("idb", [128, 128], BF16, kind="ExternalInput").ap()
    out_d = nc.dram_tensor("out", [BC, C], F32, kind="ExternalOutput").ap()
    t_scratch = nc.dram_tensor("t_scratch", [EN], F32).ap()
    if DEBUG_DUMP:
        dbg_S = nc.dram_tensor("dbg_S", [128, 4, EN], BF16, kind="ExternalOutput").ap()
        dbg_negt = nc.dram_tensor("dbg_negt", [128, EN], F32, kind="ExternalOutput").ap()
        dbg_w = nc.dram_tensor("dbg_w", [128, CH, EN], I16, kind="ExternalOutput").ap()
        dbg_p = nc.dram_tensor("dbg_p", [128, CH, EL], F32, kind="ExternalOutput").ap()
        dbg_pT = nc.dram_tensor("dbg_pT", [128, CH, BC], F32, kind="ExternalOutput").ap()
        dbg_u = nc.dram_tensor("dbg_u", [128, CH * 2, HALF], F32, kind="ExternalOutput").ap()
        dbg_ri = nc.dram_tensor("dbg_ri", [128, CH * 2, HALF], mybir.dt.int32, kind="ExternalOutput").ap()
        dbg_fl = nc.dram_tensor("dbg_fl", [128, CH * 2, HALF], F32, kind="ExternalOutput").ap()

    T_v = T_in.rearrange("e n d -> (e n) d").rearrange("(t p) d -> p t d", p=126)
    xT_v = xT_in.rearrange("(c p) b -> p c b", p=128)
    L_v = L_in.rearrange("e l c -> (e l) c").rearrange("(j p) c -> p j c", p=128)
    out_v = out_d.rearrange("(k p) c -> p k c", p=128)

    pt_dt = F32 if PT_MODE == "f32" else F32R

    with tile.TileContext(nc) as tc:
        with (
            tc.tile_pool(name="const", bufs=1) as constp,
            tc.tile_pool(name="tproc", bufs=1) as tprocp,
            tc.tile_pool(name="big", bufs=1) as bigp,
            tc.tile_pool(name="xin", bufs=2) as xinp,
            tc.tile_pool(name="work", bufs=2) as workp,
            tc.tile_pool(name="pst", bufs=2, space="PSUM") as pstp,
            tc.tile_pool(name="psu", bufs=2, space="PSUM") as psup,
            tc.tile_pool(name="psy", bufs=2, space="PSUM") as psyp,
        ):
            # ---- constants ----
            idb = constp.tile([128, 128], BF16)
            nc.sync.dma_start(idb[:], idb_in[:])
            idf = constp.tile([128, 128], F32)
            nc.sync.dma_start(idf[:], idf_in[:])
            ones = constp.tile([1, 128], F32)
            nc.vector.memset(ones[:], 1.0)
            if PT_MODE == "f32r":
                idr = constp.tile([128, 128], F32R)
                nc.vector.tensor_copy(idr[:], idf[:])

            # ---- T processing (per 126-row t-tile: 2 estimators) ----
            T_sb = tprocp.tile([126, TT, D], F32)
            tmax = tprocp.tile([126, TT], F32)
            I_sb = tprocp.tile([126, TT, D], BF16)
            for t in range(TT):
                nc.sync.dma_start(T_sb[:, t, :], T_v[:, t, :])
                nc.vector.tensor_reduce(
                    tmax[:, t : t + 1], T_sb[:, t, :],
                    axis=mybir.AxisListType.X, op=mybir.AluOpType.max,
                )
                nc.vector.tensor_scalar(
                    I_sb[:, t, :], T_sb[:, t, :], tmax[:, t : t + 1], None,
                    op0=mybir.AluOpType.is_equal,
                )

            # ---- -t broadcast tile [128, EN] ----
            # tmax [126, TT] -> DRAM [EN] -> [1, EN], negate, ones-column
            # matmul broadcast (matmul operands must sit at partition 0)
            nc.sync.dma_start(t_scratch.rearrange("(t p) -> p t", p=126), tmax[:])
            t_row = constp.tile([1, EN], F32)
            nc.sync.dma_start(t_row[:1, :], t_scratch.rearrange("(o x) -> o x", o=1))
            tb_sb = constp.tile([128, EN], F32)
            for h in range(2):
                hs = slice(h * HALF, (h + 1) * HALF)
                ntp = psup.tile([128, HALF], F32, tag="u")
                nc.tensor.matmul(
                    ntp[:], lhsT=ones[:1, :], rhs=t_row[:1, hs],
                    start=True, stop=True,
                )
                nc.scalar.activation(
                    tb_sb[:, hs], ntp[:], mybir.ActivationFunctionType.Copy
                )
            negt_v = tb_sb[:]

            # ---- S = I^T [d-part, dchunk, en] ----
            s_dt = BF16 if SEL_MODE == "pair" else F32R
            S_sb = bigp.tile([128, 4, EN], s_dt)
            for t in range(TT):
                for c in range(4):
                    tp = pstp.tile([128, 126], BF16, tag="tp")
                    nc.tensor.transpose(
                        tp[:], I_sb[:, t, c * 128 : (c + 1) * 128], idb[:126, :126]
                    )
                    nc.scalar.activation(
                        S_sb[:, c, t * 126 : (t + 1) * 126], tp[:],
                        mybir.ActivationFunctionType.Copy,
                    )

            # ---- L ----
            L_sb = constp.tile([128, CH, C], F32)
            nc.sync.dma_start(L_sb[:], L_v[:])

            # ---- x load + bf16 hi/lo split (negated), per chunk ----
            if SEL_MODE == "pair":
                xhi = bigp.tile([128, 4, BC], BF16)
                xlo = bigp.tile([128, 4, BC], BF16)
            else:
                xr = bigp.tile([128, 4, BC], F32R)
            for k in range(CH):
                ks = slice(k * 128, (k + 1) * 128)
                xc = xinp.tile([128, 4, 128], F32, tag="xc")
                nc.sync.dma_start(xc[:], xT_v[:, :, ks])
                if SEL_MODE == "pair":
                    nc.vector.tensor_scalar(
                        xhi[:, :, ks], xc[:], -1.0, None, op0=mybir.AluOpType.mult
                    )
                    nc.vector.scalar_tensor_tensor(
                        xlo[:, :, ks], xc[:], -1.0, xhi[:, :, ks],
                        op0=mybir.AluOpType.mult, op1=mybir.AluOpType.subtract,
                    )
                else:
                    nc.vector.tensor_scalar(
                        xr[:, :, ks], xc[:], -1.0, None, op0=mybir.AluOpType.mult
                    )

            # ---- per-chunk: selection + floor + tree; tail for k-1 ----
            w_sb = bigp.tile([128, CH, EN], I16)
            w4 = w_sb[:].rearrange("p k (e n) -> p k e n", n=NN)
            p_all = bigp.tile([128, CH, EL], pt_dt)
            pT = bigp.tile([128, CH, BC], F32)

            def emit_tail(k):
                ks = slice(k * 128, (k + 1) * 128)
                for jj in range(CH):
                    tp = pstp.tile([128, 128], pt_dt, tag="tp")
                    nc.tensor.transpose(
                        tp[:], p_all[:, k, jj * 128 : (jj + 1) * 128],
                        idf[:] if PT_MODE == "f32" else idr[:],
                    )
                    nc.scalar.activation(
                        pT[:, jj, ks], tp[:], mybir.ActivationFunctionType.Copy
                    )
                y_ps = psyp.tile([128, C], F32, tag="y")
                for jj in range(CH):
                    nc.tensor.matmul(
                        y_ps[:],
                        lhsT=pT[:, jj, ks],
                        rhs=L_sb[:, jj, :],
                        start=(jj == 0), stop=(jj == CH - 1),
                    )
                nm = workp.tile([128, 1], F32, tag="nm")
                nc.vector.tensor_reduce(
                    nm[:], y_ps[:], axis=mybir.AxisListType.X,
                    op=mybir.AluOpType.max, negate=True,
                )
                yexp = workp.tile([128, C], F32, tag="yexp")
                ssum = workp.tile([128, 1], F32, tag="ssum")
                nc.scalar.activation(
                    yexp[:], y_ps[:], mybir.ActivationFunctionType.Exp,
                    bias=nm[:, 0:1], scale=1.0, accum_out=ssum[:, 0:1],
                )
                rec = workp.tile([128, 1], F32, tag="rec")
                nc.vector.reciprocal(rec[:], ssum[:])
                yout = workp.tile([128, C], F32, tag="yout")
                nc.vector.tensor_scalar(
                    yout[:], yexp[:], rec[:, 0:1], None, op0=mybir.AluOpType.mult
                )
                nc.sync.dma_start(out_v[:, k, :], yout[:])

            for k in range(CH):
                ks = slice(k * 128, (k + 1) * 128)
                for h in range(2):
                    hs = slice(h * HALF, (h + 1) * HALF)
                    u_ps = psup.tile([128, HALF], F32, tag="u")
                    nc.scalar.activation(
                        u_ps[:], negt_v[:, hs], mybir.ActivationFunctionType.Copy
                    )
                    if SEL_MODE == "pair":
                        for c in range(4):
                            nc.tensor.matmul(
                                u_ps[:], lhsT=xhi[:, c, ks], rhs=S_sb[:, c, hs],
                                start=False, stop=False, skip_group_check=True,
                            )
                        for c in range(4):
                            nc.tensor.matmul(
                                u_ps[:], lhsT=xlo[:, c, ks], rhs=S_sb[:, c, hs],
                                start=False, stop=(c == 3), skip_group_check=True,
                            )
                    else:
                        for c in range(4):
                            nc.tensor.matmul(
                                u_ps[:], lhsT=xr[:, c, ks], rhs=S_sb[:, c, hs],
                                start=False, stop=(c == 3), skip_group_check=True,
                            )
                    # w = -floor(u) = -s: ACT rounds u to int32 c (either
                    # rounding mode works: floor(u) = c - [c > u]), then
                    # w = [c > u] - c on DVE (one PSUM input per op).
                    ri = workp.tile([128, HALF], mybir.dt.int32, tag="ri")
                    nc.scalar.activation(
                        ri[:], u_ps[:], mybir.ActivationFunctionType.Copy
                    )
                    flag = workp.tile([128, HALF], F32, tag="flag")
                    nc.vector.scalar_tensor_tensor(
                        flag[:], ri[:], 0.0, u_ps[:],
                        op0=mybir.AluOpType.add, op1=mybir.AluOpType.is_gt,
                    )
                    nc.vector.tensor_tensor(
                        w_sb[:, k, hs], flag[:], ri[:],
                        op=mybir.AluOpType.subtract,
                    )
                    if DEBUG_DUMP:
                        du = xinp.tile([128, HALF], F32, tag="du")
                        nc.scalar.activation(
                            du[:], u_ps[:], mybir.ActivationFunctionType.Copy
                        )
                        nc.sync.dma_start(dbg_u[:, 2 * k + h, :], du[:])
                        nc.sync.dma_start(dbg_ri[:, 2 * k + h, :], ri[:])
                        nc.sync.dma_start(dbg_fl[:, 2 * k + h, :], flag[:])

                # tree in int16; last level emits p (fp32/f32r) into p_all
                lvl = workp.tile([128, E, 2], I16, tag="lvlA")
                nc.vector.tensor_scalar(
                    lvl[:, :, 0:1], w4[:, k, :, 0:1], -1.0, None,
                    op0=mybir.AluOpType.mult,
                )
                nc.vector.tensor_scalar(
                    lvl[:, :, 1:2], w4[:, k, :, 0:1], 1.0, None,
                    op0=mybir.AluOpType.add,
                )
                for j in range(2, DEPTH + 1):
                    half = 2 ** (j - 1)
                    base = half - 1
                    if j == DEPTH:
                        nxt = p_all[:, k, :].rearrange("p (e l) -> p e l", l=NL)
                    else:
                        nxt_t = workp.tile(
                            [128, E, 2 * half], I16,
                            tag=("lvlA" if j % 2 else "lvlB"),
                        )
                        nxt = nxt_t[:]
                    nxt5 = nxt.rearrange("p e (k2 c) -> p e k2 c", c=2)
                    wj = w4[:, k, :, base : base + half]
                    par = lvl[:]
                    nc.vector.scalar_tensor_tensor(
                        nxt5[:, :, :, 0], wj, -1.0, par,
                        op0=mybir.AluOpType.mult, op1=mybir.AluOpType.mult,
                    )
                    nc.vector.scalar_tensor_tensor(
                        nxt5[:, :, :, 1], wj, 1.0, par,
                        op0=mybir.AluOpType.add, op1=mybir.AluOpType.mult,
                    )
                    if j < DEPTH:
                        lvl = nxt

                if k >= 1:
                    emit_tail(k - 1)
            emit_tail(CH - 1)

            if DEBUG_DUMP:
                if SEL_MODE == "pair":
                    nc.sync.dma_start(dbg_S[:], S_sb[:])
                nc.sync.dma_start(dbg_negt[:], tb_sb[:])
                nc.sync.dma_start(dbg_w[:], w_sb[:])
                if PT_MODE == "f32":
                    nc.sync.dma_start(dbg_p[:], p_all[:])
                nc.sync.dma_start(dbg_pT[:], pT[:])

    nc.compile()
    return nc


_id_f32 = np.eye(128, dtype=np.float32)
_id_bf16 = np.eye(128, dtype=ml_dtypes.bfloat16)


def make_in_maps(x, T, L):
    x = np.ascontiguousarray(x, dtype=np.float32)
    T = np.ascontiguousarray(T, dtype=np.float32)
    L = np.ascontiguousarray(L, dtype=np.float32)
    maps = []
    for i in range(NCORES):
        maps.append({
            "xT": np.ascontiguousarray(x[i * BC : (i + 1) * BC].T),
            "T": T,
            "L": L,
            "idf": _id_f32,
            "idb": _id_bf16,
        })
    return maps


def run(x, T, L, trace=False, **kw):
    nc = build_program()
    res = run_bass_kernel_spmd(
        nc, make_in_maps(x, T, L), core_ids=list(range(NCORES)), trace=trace, **kw
    )
    out = np.concatenate([res.results[i]["out"] for i in range(NCORES)], axis=0)
    return out, res


def kernel(x, T, L):
    out, _ = run(x, T, L, trace=False)
    return out
